# revision 47
# baseline (speedup 1.0000x reference)
"""Contrastive loss kernel for Trainium2 (8 NeuronCores).

loss = mean((sim.sum(-1) - diag) / T) with sim = n @ n.T, n = x/||x||
     = (||s||^2 - N) / (N*T)          with s = sum_i x_i / ||x_i||

Each core takes a [2048, 512] row shard laid out "(p t) d -> p (t d)":
partition p holds rows p*16..p*16+15, so tile t is the column slice
[t*512, (t+1)*512) and every DMA chunk is per-partition-contiguous.

The input streams in over two DMA queues concurrently: SP (HWDGE)
carries 6 fp32 tiles while Pool (SWDGE) carries 10 tiles cast to bf16
in-flight — the cast halves that queue's busy time, and bf16 is ample
for the row norms and the PE matmul at 2e-2 tolerance. Row
sum-of-squares is one scalar_tensor_tensor((x*1)*x, accum_out) per
tile on DVE, with a few tiles on ACT via Square+accum (its activation
table load is prepaid by a dummy sqrt while ACT is idle). rnorm =
reciprocal(sqrt(ss)) batched per group: sqrt on ACT, reciprocal on DVE,
emitted per PE dtype since the ISA forbids mixing 32/16-bit matmul
operands. The stats-engine assignment, rsqrt grouping, and DMA chunking
were tuned by randomized search against the CoreSim cost model; the
ss/rnorm tile rings use 4 buffers so consecutive rsqrt groups don't
serialize on buffer reuse, and groups pair same-dtype tiles so each
needs only one reciprocal. The
partial s_local = sum_i rnorm_i * x_i is 32 column-split PE matmuls
(fp32r/bf16, 1 cyc/row) accumulated into two [1, 256] PSUM halves:
ACT copies the earlier-finishing left half to SBUF while the final
matmuls complete, DVE copies the right half, and each half ships on
its own DMA queue (ACT, SP) so the two completion latencies overlap;
dummy warmup matmuls keep the PE p-state ramped so real matmuls run at
full clock. The host sums the 8 [1, 512] partials and applies the
scalar epilogue.
"""

import numpy as np

import concourse.bass as bass
import concourse.bacc as bacc
import concourse.tile as tile
from concourse import mybir
from concourse.bass_utils import run_bass_kernel_spmd

N = 16384
D = 512
NCORES = 8
ROWS = N // NCORES   # 2048 rows per core
P = 128              # SBUF partitions
NTILES = ROWS // P   # 16 tiles (column slices of the [128, 8192] layout)
TEMPERATURE = 0.5

F32 = mybir.dt.float32
F32R = mybir.dt.float32r
BF16 = mybir.dt.bfloat16
MULT = mybir.AluOpType.mult

# --- schedule (tuned against the CoreSim cost model) ---------------------
# DMA chunks in arrival order: (queue, n_tiles, dtype). sp/act are HWDGE
# fp32; pool is SWDGE casting to bf16.
DMA_PLAN = (
    ("pool", 1, "b"), ("sp", 1, "f"), ("sp", 1, "f"),
    ("pool", 2, "b"), ("sp", 1, "f"),
    ("pool", 2, "b"), ("sp", 1, "f"),
    ("pool", 2, "b"), ("sp", 1, "f"),
    ("pool", 2, "b"), ("sp", 1, "f"),
    ("pool", 1, "b"),
)
# stats engine per tile: d=DVE (scalar_tensor_tensor), a=ACT (Square+accum)
STATS = "ddadaddaadaddadd"
# rsqrt groups: explicit tile-index tuples (dtype-homogeneous where possible)
GROUPS = ((0, 3), (1, 2), (4, 6), (5, 8), (7, 9), (11, 14), (10, 12), (13, 15))
RECIP_ENG = "old"    # old: ACT sqrt then DVE recips | dve/pool: recip first
COPY_ENG = "act"
WARM_N = 8          # PE warmup matmuls
WARM_W = 512        # warmup matmul width


def _build_nc(dma_plan=DMA_PLAN, stats=STATS, groups=GROUPS,
              warm_n=WARM_N, warm_w=WARM_W, recip_eng=RECIP_ENG,
              copy_eng=COPY_ENG) -> bass.Bass:
    assert sum(c for _, c, _ in dma_plan) == NTILES
    assert len(stats) == NTILES
    assert sum(len(g) for g in groups) == NTILES

    nc = bacc.Bacc(None)
    x_in = nc.declare_dram_parameter("x", [ROWS, D], F32R, isOutput=False)
    s_out = nc.declare_dram_parameter("s", [1, D], F32, isOutput=True)
    # partition p <- rows p*NTILES..p*NTILES+NTILES-1 (contiguous in DRAM)
    x_t = x_in.rearrange("(p t) d -> p (t d)", p=P)

    with tile.TileContext(nc) as tc:
        with (
            tc.tile_pool(name="xs", bufs=1) as xs_pool,
            tc.tile_pool(name="wt", bufs=1) as wt_pool,
            tc.tile_pool(name="sq", bufs=2) as sq_pool,
            tc.tile_pool(name="st", bufs=4) as st_pool,
            tc.tile_pool(name="acc", bufs=1, space="PSUM") as psum_pool,
            tc.tile_pool(name="wacc", bufs=1, space="PSUM") as wpsum_pool,
            tc.tile_pool(name="one", bufs=1) as one_pool,
        ):
            accL = psum_pool.tile([1, D // 2], F32, tag="accL")
            accR = psum_pool.tile([1, D // 2], F32, tag="accR")

            # prepay the ACT activation-table load (sqrt table) while the
            # engine is otherwise idle, before its input DMAs
            dum = one_pool.tile([1, 1], F32, tag="dum")
            nc.vector.memset(dum, 1.0)
            nc.scalar.sqrt(out=dum, in_=dum)

            # PE warmup: keep the tensor engine continuously busy from t~0 so
            # its p-state ramps to full clock before the real matmuls.
            if warm_n:
                wt = wt_pool.tile([P, warm_w], BF16)
                nc.vector.memset(wt, 0.0)
                wacc = wpsum_pool.tile([1, warm_w], F32)
                for _ in range(warm_n):
                    nc.tensor.matmul(wacc, lhsT=wt[:, 0:1], rhs=wt[:, :],
                                     start=True, stop=True)

            # input DMA chunks; per-queue busy is bytes-based, transfers on
            # different queues overlap
            queues = {"sp": nc.sync, "act": nc.scalar, "pool": nc.gpsimd}
            xmm = [None] * NTILES   # PE operand views (f32r or bf16)
            xst = [None] * NTILES   # stats views (f32 or bf16)
            t0 = 0
            for qname, ctiles, dt in dma_plan:
                w = ctiles * D
                if dt == "b":
                    xb = xs_pool.tile([P, w], BF16, tag=f"x{t0}")
                    queues[qname].dma_start(out=xb, in_=x_t[:, t0 * D : t0 * D + w])
                    for j in range(ctiles):
                        v = xb[:, j * D : (j + 1) * D]
                        xmm[t0 + j] = v
                        xst[t0 + j] = v
                else:
                    xb = xs_pool.tile([P, w], F32R, tag=f"x{t0}")
                    queues[qname].dma_start(out=xb, in_=x_t[:, t0 * D : t0 * D + w])
                    for j in range(ctiles):
                        v = xb[:, j * D : (j + 1) * D]
                        xmm[t0 + j] = v
                        xst[t0 + j] = v.bitcast(F32)
                t0 += ctiles

            def emit_stats(eng, t, ss_col):
                dt = xst[t].dtype
                sq = sq_pool.tile([P, D], dt, tag=f"sq_{eng}_{dt}")
                if eng == "a":
                    # ACT: ss = sum_d x^2 via Square + accumulate
                    nc.scalar.activation(
                        out=sq, in_=xst[t].bitcast(F32) if dt == F32R else xst[t],
                        func=mybir.ActivationFunctionType.Square, accum_out=ss_col,
                    )
                elif eng == "p":
                    # Pool: square then reduce (gpsimd has no fused accum)
                    nc.gpsimd.tensor_mul(sq, xst[t], xst[t])
                    nc.gpsimd.tensor_reduce(
                        out=ss_col, in_=sq, axis=mybir.AxisListType.X,
                        op=mybir.AluOpType.add,
                    )
                else:
                    # DVE: ss = sum((x*1)*x) in one scalar_tensor_tensor
                    nc.vector.scalar_tensor_tensor(
                        out=sq, in0=xst[t], scalar=1.0, in1=xst[t],
                        op0=MULT, op1=MULT, accum_out=ss_col,
                    )

            emitted = 0
            first_mm = True
            last_t = groups[-1][-1]
            for gi, tiles in enumerate(groups):
                last_group = gi == len(groups) - 1
                gsz = len(tiles)
                ss = st_pool.tile([P, gsz], F32, tag="ss")
                for j, t in enumerate(tiles):
                    emit_stats(stats[t], t, ss[:, j : j + 1])
                # rnorm = 1/sqrt(ss): reciprocal first (Pool normalize_recip
                # in place, or DVE), then sqrt on ACT writing the PE dtype.
                dts = {xmm[t].dtype for t in tiles}
                r = {}
                if last_group and all(stats[t] == "d" for t in tiles):
                    # recip first on DVE (same queue as the stats - no hop),
                    # then per-dtype sqrt on ACT
                    nc.vector.reciprocal(out=ss, in_=ss)
                    with nc.allow_low_precision(reason="PE operand rounding"):
                        for dt in dts:
                            rt = st_pool.tile([P, gsz], dt, tag=f"rn{dt}")
                            nc.scalar.sqrt(out=rt, in_=ss)
                            r[dt] = rt
                elif recip_eng == "old":
                    # sqrt in place on ACT, then per-dtype DVE recips
                    nc.scalar.sqrt(out=ss, in_=ss)
                    with nc.allow_low_precision(reason="PE operand rounding"):
                        for dt in dts:
                            rt = st_pool.tile([P, gsz], dt, tag=f"rn{dt}")
                            nc.vector.reciprocal(out=rt, in_=ss)
                            r[dt] = rt
                else:
                    if recip_eng == "pool":
                        for j in range(gsz):
                            nrd = sq_pool.tile([P, 1], F32, tag="nrd")
                            nc.gpsimd.normalize_recip(
                                out_ap=nrd,
                                in_ap=ss[:, j : j + 1],
                                denom_ap=ss[:, j : j + 1],
                            )
                    else:
                        nc.vector.reciprocal(out=ss, in_=ss)
                    with nc.allow_low_precision(reason="PE operand rounding"):
                        for dt in dts:
                            rt = st_pool.tile([P, gsz], dt, tag=f"rn{dt}")
                            nc.scalar.sqrt(out=rt, in_=ss)
                            r[dt] = rt
                for j, t in enumerate(tiles):
                    # column-split: the left half finishes one half-matmul
                    # early so its PSUM copy overlaps the final matmuls
                    nc.tensor.matmul(
                        accL,
                        lhsT=r[xmm[t].dtype][:, j : j + 1],
                        rhs=xmm[t][:, 0 : D // 2],
                        start=first_mm,
                        stop=(t == last_t),
                    )
                    nc.tensor.matmul(
                        accR,
                        lhsT=r[xmm[t].dtype][:, j : j + 1],
                        rhs=xmm[t][:, D // 2 :],
                        start=first_mm,
                        stop=(t == last_t),
                    )
                    first_mm = False
                emitted += gsz

            res = one_pool.tile([1, D], F32)
            # ACT copies the earlier-finishing left half; DVE (faster copy)
            # takes the right half that completes with the final matmul
            nc.scalar.copy(out=res[:, 0 : D // 2], in_=accL)
            nc.vector.tensor_copy(res[:, D // 2 :], accR)
            # split output DMA so the two 1717ns completion latencies overlap
            nc.scalar.dma_start(out=s_out[:, 0 : D // 2], in_=res[:, 0 : D // 2])
            nc.sync.dma_start(out=s_out[:, D // 2 :], in_=res[:, D // 2 :])

    nc.finalize()
    return nc


_NC = None


def _run(x: np.ndarray, trace: bool = False):
    global _NC
    if _NC is None:
        _NC = _build_nc()
    x = np.ascontiguousarray(np.asarray(x, dtype=np.float32)).reshape(NCORES, ROWS, D)
    in_maps = [{"x": x[c]} for c in range(NCORES)]
    out = run_bass_kernel_spmd(_NC, in_maps, core_ids=list(range(NCORES)), trace=trace)
    s = np.zeros(D, dtype=np.float64)
    for r in out.results:
        s += r["s"].reshape(D).astype(np.float64)
    loss = (float(s @ s) - float(N)) / (N * TEMPERATURE)
    return np.asarray(loss, dtype=np.float32), out


def kernel(x: np.ndarray) -> np.ndarray:
    loss, _ = _run(x)
    return loss
